# revision 23
# baseline (speedup 1.0000x reference)
import contextlib
import numpy as np
import ml_dtypes

import concourse.bass as bass
import concourse.mybir as mybir
import concourse.tile as tile
from concourse import bacc
from concourse.masks import make_identity

bf16 = ml_dtypes.bfloat16
FP32 = mybir.dt.float32
BF16 = mybir.dt.bfloat16
AF = mybir.ActivationFunctionType

B, T, C, H = 2, 2048, 1024, 16
HS = C // H
NCORES = 8
HPC = H // NCORES
TOK = B * T
EPS = 1e-5
CT = C // 128
NCH = TOK // 512
QB = 512
ROWS = TOK // NCORES
HID = 4 * C
HT = HID // 128
MT = ROWS // 128


def _emit_attnv(nc, vts, vcol, item):
    pasl, koff, pr, c0, is_start, is_stop = item
    nc.tensor.matmul(pasl[:, c0:] if c0 else pasl, vts[koff // 128][:, vcol],
                     pr[:, c0:] if c0 else pr,
                     start=is_start, stop=is_stop, skip_group_check=True)


def build_l1(debug=False, use_beta=True):
    nc = bacc.Bacc("TRN2", target_bir_lowering=False, debug=False, num_devices=NCORES)
    xt_d = nc.dram_tensor("xt", [C, TOK], BF16, kind="ExternalInput").ap()
    wq_d = nc.dram_tensor("wq", [C, 128], BF16, kind="ExternalInput").ap()
    wk_d = nc.dram_tensor("wk", [C, 128], BF16, kind="ExternalInput").ap()
    wv_d = nc.dram_tensor("wv", [C, 128], BF16, kind="ExternalInput").ap()
    nws_d = nc.dram_tensor("nws", [128, 3], FP32, kind="ExternalInput").ap()
    wb_d = nc.dram_tensor("wb", [128, 3], FP32, kind="ExternalInput").ap()
    tri_d = nc.dram_tensor("tri", [128, 128], BF16, kind="ExternalInput").ap()
    out_d = nc.dram_tensor("attn_out", [128, TOK], BF16, kind="ExternalOutput").ap()
    if debug:
        dbg_rstd = nc.dram_tensor("dbg_rstd", [128, TOK], FP32, kind="ExternalOutput").ap()
        dbg_mur = nc.dram_tensor("dbg_mur", [128, TOK], FP32, kind="ExternalOutput").ap()
        dbg_qt = nc.dram_tensor("dbg_qt", [128, TOK], BF16, kind="ExternalOutput").ap()
        dbg_kt = nc.dram_tensor("dbg_kt", [128, TOK], BF16, kind="ExternalOutput").ap()
        dbg_v = nc.dram_tensor("dbg_v", [128, 130], BF16, kind="ExternalOutput").ap()

    with tile.TileContext(nc) as tc, contextlib.ExitStack() as ctx:
        consts = ctx.enter_context(tc.tile_pool(name="consts", bufs=1))
        hpool = ctx.enter_context(tc.tile_pool(name="hT", bufs=1))
        stats = ctx.enter_context(tc.tile_pool(name="stats", bufs=1))
        tmp = ctx.enter_context(tc.tile_pool(name="tmp", bufs=3))
        qkv = ctx.enter_context(tc.tile_pool(name="qkv", bufs=1))
        probs_p = ctx.enter_context(tc.tile_pool(name="probs", bufs=16))
        attn_sb_p = ctx.enter_context(tc.tile_pool(name="attn_sb", bufs=3))
        ps_rot = ctx.enter_context(tc.tile_pool(name="ps_rot", bufs=5, space="PSUM"))
        ps_acc = ctx.enter_context(tc.tile_pool(name="ps_acc", bufs=3, space="PSUM"))

        ones_sb = consts.tile([128, 128], BF16)
        nc.vector.memset(ones_sb, 1.0)
        eps_sb = consts.tile([128, 1], FP32)
        nc.vector.memset(eps_sb, EPS)
        ident = consts.tile([128, 128], BF16)
        make_identity(nc, ident)

        warm_ps = ps_acc.tile([128, 512], FP32, tag="pa")
        for _ in range(48):
            nc.tensor.matmul(warm_ps[:, 0:128], ones_sb, ones_sb[:, 0:128], start=True, stop=True)

        wq_sb = consts.tile([128, CT, 128], BF16)
        nc.sync.dma_start(out=wq_sb, in_=wq_d.rearrange("(a p) m -> p a m", p=128))
        wk_sb = consts.tile([128, CT, 128], BF16)
        nc.sync.dma_start(out=wk_sb, in_=wk_d.rearrange("(a p) m -> p a m", p=128))
        wv_sb = consts.tile([128, CT, 128], BF16)
        nc.sync.dma_start(out=wv_sb, in_=wv_d.rearrange("(a p) m -> p a m", p=128))
        nws_sb = consts.tile([128, 3], FP32)
        nc.sync.dma_start(out=nws_sb, in_=nws_d)
        wb_sb = consts.tile([128, 3], FP32)
        nc.sync.dma_start(out=wb_sb, in_=wb_d)
        tri_sb = consts.tile([128, 128], BF16)
        nc.sync.dma_start(out=tri_sb, in_=tri_d)

        xts = []
        for ci in range(CT):
            t = hpool.tile([128, TOK], BF16, tag=f"hT{ci}")
            xts.append(t)
        for j in range(NCH):
            sl = slice(j * 512, (j + 1) * 512)
            for ci in range(CT):
                nc.sync.dma_start(out=xts[ci][:, sl], in_=xt_d[ci * 128:(ci + 1) * 128, sl])

        rstd_b = stats.tile([128, TOK], BF16, tag="rstd_b")
        murstd_b = stats.tile([128, TOK], BF16, tag="murstd_b")
        mu_b = stats.tile([128, TOK], FP32, tag="mu_b")
        var_b = stats.tile([128, TOK], FP32, tag="var_b")
        for j in range(NCH):
            sl = slice(j * 512, (j + 1) * 512)
            ps_sum = ps_rot.tile([128, 512], FP32, tag="mm")
            ps_sq = ps_rot.tile([128, 512], FP32, tag="mm")
            for ci in range(CT):
                sq = tmp.tile([128, 512], BF16, tag="sq")
                nc.vector.tensor_mul(sq, xts[ci][:, sl], xts[ci][:, sl])
                nc.tensor.matmul(ps_sum, ones_sb, xts[ci][:, sl],
                                 start=(ci == 0), stop=(ci == CT - 1))
                nc.tensor.matmul(ps_sq, ones_sb, sq,
                                 start=(ci == 0), stop=(ci == CT - 1))
            nc.scalar.mul(out=mu_b[:, sl], in_=ps_sum, mul=1.0 / C)
            nc.scalar.mul(out=var_b[:, sl], in_=ps_sq, mul=1.0 / C)
            mu2 = tmp.tile([128, 512], BF16, tag="mu2")
            nc.vector.tensor_mul(mu2, mu_b[:, sl], mu_b[:, sl])
            nc.vector.tensor_sub(var_b[:, sl], var_b[:, sl], mu2)
        nc.scalar.activation(out=var_b, in_=var_b, func=AF.Ln, bias=eps_sb, scale=1.0)
        nc.scalar.activation(out=rstd_b, in_=var_b, func=AF.Exp, scale=-0.5)
        nc.vector.tensor_mul(murstd_b, mu_b, rstd_b)

        qt_sb = qkv.tile([128, TOK], BF16, tag="qt")
        kt_sb = qkv.tile([128, TOK], BF16, tag="kt")
        vt_sb = qkv.tile([128, TOK], BF16, tag="vt")
        wtriples = ((wq_sb, 0, qt_sb), (wk_sb, 1, kt_sb), (wv_sb, 2, vt_sb))

        def qkv_fold(tsb, idx, sl, ps):
            nc.scalar.copy(out=tsb[:, sl], in_=ps)
            nc.vector.tensor_mul(tsb[:, sl], tsb[:, sl], rstd_b[:, sl])
            nc.vector.scalar_tensor_tensor(
                tsb[:, sl], murstd_b[:, sl], nws_sb[:, idx:idx + 1], tsb[:, sl],
                op0=mybir.AluOpType.mult, op1=mybir.AluOpType.add)
            if use_beta:
                nc.vector.tensor_scalar_add(tsb[:, sl], tsb[:, sl],
                                            wb_sb[:, idx:idx + 1])

        def vt_build(tt):
            vt = vts[tt]
            ptv = ps_rot.tile([128, 128], BF16, tag="mm")
            nc.tensor.transpose(ptv, vt_sb[:, tt * 128:(tt + 1) * 128], ident)
            nc.scalar.copy(out=vt.rearrange("p (g c) -> p g c", g=2)[:, :, 0:64],
                           in_=ptv.rearrange("p (g c) -> p g c", g=2))
            nc.vector.memset(vt[:, 64:65], 1.0)
            nc.vector.memset(vt[:, 129:130], 1.0)

        vts = []
        for tt in range(TOK // 128):
            vt_tile = qkv.tile([128, 130], BF16, tag=f"v{tt}", name=f"v{tt}")
            vts.append(vt_tile)

        for wsb, idx, tsb in wtriples:
            pss = []
            for jl in range(4):
                p = ps_rot.tile([128, 512], FP32, tag="mm")
                pss.append(p)
            for ci in range(CT):
                for jl in range(4):
                    sl = slice(jl * 512, (jl + 1) * 512)
                    nc.tensor.matmul(pss[jl], wsb[:, ci, :], xts[ci][:, sl],
                                     start=(ci == 0), stop=(ci == CT - 1))
            for jl in range(4):
                qkv_fold(tsb, idx, slice(jl * 512, (jl + 1) * 512), pss[jl])
        for tt in range(16):
            vt_build(tt)

        def late_work():
            for wsb, idx, tsb in wtriples:
                for jp in range(2):
                    ps_a = ps_rot.tile([128, 512], FP32, tag="mm")
                    ps_b = ps_rot.tile([128, 512], FP32, tag="mm")
                    j0 = 4 + 2 * jp
                    sl_a = slice(j0 * 512, (j0 + 1) * 512)
                    sl_b = slice((j0 + 1) * 512, (j0 + 2) * 512)
                    for ci in range(CT):
                        nc.tensor.matmul(ps_a, wsb[:, ci, :], xts[ci][:, sl_a],
                                         start=(ci == 0), stop=(ci == CT - 1),
                                         skip_group_check=True)
                        nc.tensor.matmul(ps_b, wsb[:, ci, :], xts[ci][:, sl_b],
                                         start=(ci == 0), stop=(ci == CT - 1),
                                         skip_group_check=True)
                        yield
                    qkv_fold(tsb, idx, sl_a, ps_a)
                    qkv_fold(tsb, idx, sl_b, ps_b)
                    yield
            for tt in range(16, 32):
                vt_build(tt)
                yield
            ready_b1[0] = True

        if debug:
            nc.sync.dma_start(out=dbg_rstd, in_=rstd_b)
            nc.sync.dma_start(out=dbg_mur, in_=murstd_b)
            nc.sync.dma_start(out=dbg_qt, in_=qt_sb)
            nc.sync.dma_start(out=dbg_kt, in_=kt_sb)
            nc.sync.dma_start(out=dbg_v, in_=vts[0])

        scale = C ** -0.5

        def attn_group(b, hl, gate=None):
            hsl = slice(hl * 64, (hl + 1) * 64)
            vcol = slice(hl * 65, hl * 65 + 65)
            pend = []

            def _flush(item):
                pa_, q0_, koff_, pr_, c0_, st_, sp_ = item
                nc.tensor.matmul(pa_[:, c0_:] if c0_ else pa_,
                                 vts[koff_ // 128][:, vcol],
                                 pr_[:, c0_:] if c0_ else pr_,
                                 start=st_, stop=sp_, skip_group_check=True)
                if sp_:
                    asb = attn_sb_p.tile([65, 512], FP32, tag="asb")
                    nc.scalar.copy(out=asb, in_=pa_)
                    den = attn_sb_p.tile([1, 512], FP32, tag="den")
                    nc.gpsimd.dma_start(out=den, in_=asb[64:65, :])
                    rec = attn_sb_p.tile([64, 512], FP32, tag="rec")
                    nc.gpsimd.partition_broadcast(rec, den, channels=64)
                    recf = attn_sb_p.tile([64, 512], FP32, tag="recf")
                    nc.vector.reciprocal_approx_fast(recf, rec)
                    ao = attn_sb_p.tile([64, 512], BF16, tag="ao")
                    nc.vector.tensor_mul(ao, asb[0:64, :], recf)
                    nc.gpsimd.dma_start(out=out_d[hl * 64:(hl + 1) * 64, q0_:q0_ + QB],
                                         in_=ao)

            if gate is not None:
                while not gate[0]:
                    yield
            for j in range(T // QB):
                q0 = b * T + j * QB
                pa = ps_acc.tile([65, 512], FP32, tag="pa")
                nkt = 4 * (j + 1)
                for kt in range(nkt):
                    koff = b * T + kt * 128
                    d = kt - 4 * j
                    c0 = 128 * d if d > 0 else 0
                    ps = ps_rot.tile([128, 512], FP32, tag="mm")
                    nc.tensor.matmul(ps[:, c0:], kt_sb[hsl, koff:koff + 128],
                                     qt_sb[hsl, q0 + c0:q0 + QB],
                                     start=True, stop=True)
                    pr = probs_p.tile([128, 512], BF16, tag="pr")
                    nc.scalar.activation(out=pr[:, c0:], in_=ps[:, c0:],
                                         func=AF.Exp, scale=scale)
                    if d >= 0:
                        nc.vector.tensor_mul(pr[:, 128 * d:128 * (d + 1)],
                                             pr[:, 128 * d:128 * (d + 1)], tri_sb)
                    pend.append((pa, q0, koff, pr, c0, kt == 0, kt == nkt - 1))
                    if len(pend) > 4:
                        _flush(pend.pop(0))
                    yield
            while pend:
                _flush(pend.pop(0))
                yield

        ready_b1 = [False]
        gens = [attn_group(0, 0), attn_group(0, 1), late_work(),
                attn_group(1, 0, ready_b1), attn_group(1, 1, ready_b1)]
        while gens:
            for g in list(gens):
                try:
                    next(g)
                except StopIteration:
                    gens.remove(g)
    nc.compile()
    return nc


FP8 = mybir.dt.float8e4
f8 = ml_dtypes.float8_e4m3


def build_l2():
    nc = bacc.Bacc("TRN2", target_bir_lowering=False, debug=False, num_devices=NCORES)
    atq_d = nc.dram_tensor("atq", [C, ROWS], BF16, kind="ExternalInput").ap()
    wpq_d = nc.dram_tensor("wpq", [128, CT, CT, 128], BF16, kind="ExternalInput").ap()
    xr_d = nc.dram_tensor("xrt", [C, ROWS], FP32, kind="ExternalInput").ap()
    w1q_d = nc.dram_tensor("w1q", [128, HT, CT * 128], BF16, kind="ExternalInput").ap()
    w2q_d = nc.dram_tensor("w2q", [128, HT, C], BF16, kind="ExternalInput").ap()
    b1_d = nc.dram_tensor("b1r", [128, HT], FP32, kind="ExternalInput").ap()
    b2_d = nc.dram_tensor("b2c", [128, CT], FP32, kind="ExternalInput").ap()
    out_d = nc.dram_tensor("outT", [C, ROWS], FP32, kind="ExternalOutput").ap()

    with tile.TileContext(nc) as tc, contextlib.ExitStack() as ctx:
        consts = ctx.enter_context(tc.tile_pool(name="consts", bufs=1))
        persist = ctx.enter_context(tc.tile_pool(name="persist", bufs=1))
        wstream = ctx.enter_context(tc.tile_pool(name="wstream", bufs=4))
        tmp = ctx.enter_context(tc.tile_pool(name="tmp", bufs=2))
        small = ctx.enter_context(tc.tile_pool(name="small", bufs=1))
        ps_main = ctx.enter_context(tc.tile_pool(name="ps_main", bufs=6, space="PSUM"))
        ps_stats = ctx.enter_context(tc.tile_pool(name="ps_stats", bufs=1, space="PSUM"))

        ones_w = consts.tile([128, 128], BF16)
        nc.vector.memset(ones_w, 1.0)
        dummy_m = consts.tile([128, 512], BF16)
        nc.vector.memset(dummy_m, 0.0)
        eps_sb = consts.tile([128, 1], FP32)
        nc.vector.memset(eps_sb, EPS)

        warm_ps = ps_main.tile([128, 512], FP32, tag="mm")
        for _ in range(24):
            nc.tensor.matmul(warm_ps[:, 0:128], ones_w, ones_w[:, 0:128],
                             start=True, stop=True)

        atq_sb = consts.tile([128, CT, ROWS], BF16)
        nc.sync.dma_start(out=atq_sb,
                          in_=atq_d.rearrange("(a p) m -> p a m", p=128))
        wpq_sb = []
        for n8 in range(CT):
            wpt = consts.tile([128, CT, 128], BF16, tag=f"wpq{n8}", name=f"wpq{n8}")
            eng = nc.sync if n8 % 2 == 0 else nc.gpsimd
            eng.dma_start(out=wpt, in_=wpq_d[:, n8])
            wpq_sb.append(wpt)
        xr_sb = consts.tile([128, CT, ROWS], FP32)
        nc.scalar.dma_start(out=xr_sb,
                            in_=xr_d.rearrange("(a p) m -> p a m", p=128))
        b1_sb = consts.tile([128, HT], FP32)
        nc.scalar.dma_start(out=b1_sb, in_=b1_d)
        b2_sb = consts.tile([128, CT], FP32)
        nc.scalar.dma_start(out=b2_sb, in_=b2_d)
        w2_sb = consts.tile([128, HT, C], BF16)
        for g in range(8):
            nc.scalar.dma_start(out=w2_sb[:, g * 4:(g + 1) * 4],
                                in_=w2q_d[:, g * 4:(g + 1) * 4])

        x2_sb = persist.tile([128, CT, ROWS], FP32, tag="x2")
        ps_sum = ps_stats.tile([128, 512], FP32, tag="sum")
        ps_sq = ps_stats.tile([128, 512], FP32, tag="sq")
        for n8 in range(CT):
            pp = ps_main.tile([128, 512], FP32, tag="mm")
            for ci in range(CT):
                nc.tensor.matmul(pp, wpq_sb[n8][:, ci],
                                 atq_sb[:, ci], start=(ci == 0), stop=(ci == CT - 1),
                                 skip_group_check=True)
            nc.vector.tensor_add(x2_sb[:, n8], pp, xr_sb[:, n8])
            x2b = tmp.tile([128, 512], BF16, tag="x2b")
            nc.scalar.copy(out=x2b, in_=x2_sb[:, n8])
            sqb = tmp.tile([128, 512], BF16, tag="sqb")
            nc.vector.tensor_mul(sqb, x2b, x2b)
            nc.tensor.matmul(ps_sum, ones_w, x2b,
                             start=(n8 == 0), stop=(n8 == CT - 1),
                             skip_group_check=True)
            nc.tensor.matmul(ps_sq, ones_w, sqb,
                             start=(n8 == 0), stop=(n8 == CT - 1),
                             skip_group_check=True)

        fill_ps = ps_main.tile([128, 512], FP32, tag="mm")
        for _ in range(32):
            nc.tensor.matmul(fill_ps, ones_w, dummy_m, start=True, stop=True,
                             skip_group_check=True)

        mu = small.tile([128, 512], FP32, tag="mu")
        nc.scalar.mul(out=mu, in_=ps_sum, mul=1.0 / C)
        var = small.tile([128, 512], FP32, tag="var")
        nc.scalar.mul(out=var, in_=ps_sq, mul=1.0 / C)
        mu2 = small.tile([128, 512], FP32, tag="mu2")
        nc.vector.tensor_mul(mu2, mu, mu)
        nc.vector.tensor_sub(var, var, mu2)
        nc.scalar.activation(out=var, in_=var, func=AF.Ln, bias=eps_sb, scale=1.0)
        rstd = small.tile([128, 512], FP32, tag="rstd")
        nc.scalar.activation(out=rstd, in_=var, func=AF.Exp, scale=-0.5)
        murstd = small.tile([128, 512], FP32, tag="murstd")
        nc.vector.tensor_mul(murstd, mu, rstd)

        h2_sb = persist.tile([128, CT, ROWS], BF16, tag="h2")
        for ci in range(CT):
            th = tmp.tile([128, 512], FP32, tag="th")
            eng = nc.gpsimd if ci >= 6 else nc.vector
            eng.tensor_mul(th, x2_sb[:, ci], rstd)
            eng.tensor_sub(h2_sb[:, ci], th, murstd)

        h1_sb = persist.tile([128, HT, ROWS], BF16, tag="h1")
        for ht in range(HT):
            w1t = wstream.tile([128, CT, 128], BF16, tag="w1t")
            weng = nc.sync if ht % 2 == 0 else nc.gpsimd
            weng.dma_start(out=w1t,
                           in_=w1q_d[:, ht, :].rearrange("p (a m) -> p a m", a=CT))
            ph = ps_main.tile([128, 512], FP32, tag="mm")
            for ci in range(CT):
                nc.tensor.matmul(ph, w1t[:, ci], h2_sb[:, ci],
                                 start=(ci == 0), stop=(ci == CT - 1),
                                 skip_group_check=True)
            nc.scalar.activation(out=h1_sb[:, ht], in_=ph,
                                 func=AF.Relu, bias=b1_sb[:, ht:ht + 1], scale=1.0)

        for pn in range(4):
            pos = []
            for k in range(2):
                po = ps_main.tile([128, 512], FP32, tag="mm")
                pos.append(po)
            for ht in range(HT):
                for k in range(2):
                    n8 = pn * 2 + k
                    nc.tensor.matmul(pos[k], w2_sb[:, ht, n8 * 128:(n8 + 1) * 128],
                                     h1_sb[:, ht], start=(ht == 0), stop=(ht == HT - 1),
                                     skip_group_check=True)
            for k in range(2):
                n8 = pn * 2 + k
                td = tmp.tile([128, 512], FP32, tag="td")
                nc.scalar.activation(out=td, in_=pos[k], func=AF.Identity,
                                     bias=b2_sb[:, n8:n8 + 1], scale=1.0)
                ot = tmp.tile([128, 512], FP32, tag="ot")
                nc.vector.tensor_add(ot, td, x2_sb[:, n8])
                nc.sync.dma_start(out=out_d[n8 * 128:(n8 + 1) * 128, :], in_=ot)
    nc.compile()
    return nc



def prep_l1_inputs(inputs):
    x = np.asarray(inputs["x"], np.float32).reshape(TOK, C)
    g1 = np.asarray(inputs["g1"], np.float32)
    beta1 = np.asarray(inputs["beta1"], np.float32)
    xt = np.ascontiguousarray(x.T).astype(bf16)
    wq = (g1[:, None] * np.asarray(inputs["Wq"], np.float32)).astype(bf16)
    wk = (g1[:, None] * np.asarray(inputs["Wk"], np.float32)).astype(bf16)
    wv = (g1[:, None] * np.asarray(inputs["Wv"], np.float32)).astype(bf16)
    tri = np.triu(np.ones((128, 128), np.float32)).astype(bf16)
    in_maps = []
    for c in range(NCORES):
        csl = slice(c * 128, (c + 1) * 128)
        nws = np.stack([-wq[:, csl].astype(np.float32).sum(0),
                        -wk[:, csl].astype(np.float32).sum(0),
                        -wv[:, csl].astype(np.float32).sum(0)], axis=1)
        wb = np.stack([wq[:, csl].astype(np.float32).T @ beta1,
                       wk[:, csl].astype(np.float32).T @ beta1,
                       wv[:, csl].astype(np.float32).T @ beta1], axis=1)
        in_maps.append({
            "xt": xt,
            "wq": np.ascontiguousarray(wq[:, csl]),
            "wk": np.ascontiguousarray(wk[:, csl]),
            "wv": np.ascontiguousarray(wv[:, csl]),
            "nws": np.ascontiguousarray(nws.astype(np.float32)),
            "wb": np.ascontiguousarray(wb.astype(np.float32)),
            "tri": tri,
        })
    return in_maps


def prep_l2_inputs(inputs, attn_t):
    attn_t = np.ascontiguousarray(np.asarray(attn_t, bf16))
    x = np.asarray(inputs["x"], np.float32).reshape(TOK, C)
    x = x + np.asarray(inputs["bp"], np.float32)[None, :]
    g2 = np.asarray(inputs["g2"], np.float32)
    beta2 = np.asarray(inputs["beta2"], np.float32)
    wp = np.asarray(inputs["Wp"], np.float32)
    w1 = np.asarray(inputs["W1"], np.float32)
    w2 = np.asarray(inputs["W2"], np.float32)
    wpq = np.ascontiguousarray(
        wp.astype(bf16).reshape(CT, 128, CT, 128).transpose(1, 2, 0, 3))
    w1g = (g2[:, None] * w1).astype(bf16)
    w1q = np.ascontiguousarray(
        w1g.reshape(CT, 128, HT, 128).transpose(1, 2, 0, 3).reshape(128, HT, CT * 128))
    w2q = np.ascontiguousarray(
        w2.astype(bf16).reshape(HT, 128, C).transpose(1, 0, 2))
    b1e = np.asarray(inputs["b1"], np.float32) + w1.T @ beta2
    b1r = np.ascontiguousarray(b1e.reshape(HT, 128).T)
    b2c = np.ascontiguousarray(
        np.asarray(inputs["b2"], np.float32).reshape(CT, 128).T)
    in_maps = []
    for c in range(NCORES):
        rsl = slice(c * ROWS, (c + 1) * ROWS)
        in_maps.append({
            "atq": np.ascontiguousarray(attn_t[:, rsl]),
            "wpq": wpq,
            "xrt": np.ascontiguousarray(x[rsl, :].T),
            "w1q": w1q,
            "w2q": w2q,
            "b1r": b1r,
            "b2c": b2c,
        })
    return in_maps


_CACHE = {}


def _get_programs(use_beta):
    key = ("progs", bool(use_beta))
    if key not in _CACHE:
        nc1 = build_l1(use_beta=use_beta)
        nc2 = build_l2()
        _CACHE[key] = (nc1, nc2)
    return _CACHE[key]


def kernel(**inputs):
    from concourse.bass_utils import run_bass_kernel_spmd

    inputs = {k: np.asarray(v) for k, v in inputs.items()}
    use_beta = bool(np.any(np.asarray(inputs["beta1"], np.float32) != 0.0))
    nc1, nc2 = _get_programs(use_beta)
    core_ids = list(range(NCORES))

    r1 = run_bass_kernel_spmd(nc1, prep_l1_inputs(inputs), core_ids)
    attn_t = np.concatenate(
        [np.asarray(r1.results[c]["attn_out"]) for c in range(NCORES)], axis=0)

    r2 = run_bass_kernel_spmd(nc2, prep_l2_inputs(inputs, attn_t), core_ids)
    out = np.concatenate(
        [np.asarray(r2.results[c]["outT"]).T for c in range(NCORES)], axis=0)
    return np.ascontiguousarray(out.reshape(B, T, C).astype(np.float32))



# revision 24
# speedup vs baseline: 1.0095x; 1.0095x over previous
import contextlib
import numpy as np
import ml_dtypes

import concourse.bass as bass
import concourse.mybir as mybir
import concourse.tile as tile
from concourse import bacc
from concourse.masks import make_identity

bf16 = ml_dtypes.bfloat16
FP32 = mybir.dt.float32
BF16 = mybir.dt.bfloat16
AF = mybir.ActivationFunctionType

B, T, C, H = 2, 2048, 1024, 16
HS = C // H
NCORES = 8
HPC = H // NCORES
TOK = B * T
EPS = 1e-5
CT = C // 128
NCH = TOK // 512
QB = 512
ROWS = TOK // NCORES
HID = 4 * C
HT = HID // 128
MT = ROWS // 128


def _emit_attnv(nc, vts, vcol, item):
    pasl, koff, pr, c0, is_start, is_stop = item
    nc.tensor.matmul(pasl[:, c0:] if c0 else pasl, vts[koff // 128][:, vcol],
                     pr[:, c0:] if c0 else pr,
                     start=is_start, stop=is_stop, skip_group_check=True)


def build_l1(debug=False, use_beta=True):
    nc = bacc.Bacc("TRN2", target_bir_lowering=False, debug=False, num_devices=NCORES)
    xt_d = nc.dram_tensor("xt", [C, TOK], BF16, kind="ExternalInput").ap()
    wq_d = nc.dram_tensor("wq", [C, 128], BF16, kind="ExternalInput").ap()
    wk_d = nc.dram_tensor("wk", [C, 128], BF16, kind="ExternalInput").ap()
    wv_d = nc.dram_tensor("wv", [C, 128], BF16, kind="ExternalInput").ap()
    nws_d = nc.dram_tensor("nws", [128, 3], FP32, kind="ExternalInput").ap()
    wb_d = nc.dram_tensor("wb", [128, 3], FP32, kind="ExternalInput").ap()
    tri_d = nc.dram_tensor("tri", [128, 128], BF16, kind="ExternalInput").ap()
    out_d = nc.dram_tensor("attn_out", [128, TOK], BF16, kind="ExternalOutput").ap()
    if debug:
        dbg_rstd = nc.dram_tensor("dbg_rstd", [128, TOK], FP32, kind="ExternalOutput").ap()
        dbg_mur = nc.dram_tensor("dbg_mur", [128, TOK], FP32, kind="ExternalOutput").ap()
        dbg_qt = nc.dram_tensor("dbg_qt", [128, TOK], BF16, kind="ExternalOutput").ap()
        dbg_kt = nc.dram_tensor("dbg_kt", [128, TOK], BF16, kind="ExternalOutput").ap()
        dbg_v = nc.dram_tensor("dbg_v", [128, 130], BF16, kind="ExternalOutput").ap()

    with tile.TileContext(nc) as tc, contextlib.ExitStack() as ctx:
        consts = ctx.enter_context(tc.tile_pool(name="consts", bufs=1))
        hpool = ctx.enter_context(tc.tile_pool(name="hT", bufs=1))
        stats = ctx.enter_context(tc.tile_pool(name="stats", bufs=1))
        tmp = ctx.enter_context(tc.tile_pool(name="tmp", bufs=3))
        qkv = ctx.enter_context(tc.tile_pool(name="qkv", bufs=1))
        probs_p = ctx.enter_context(tc.tile_pool(name="probs", bufs=16))
        attn_sb_p = ctx.enter_context(tc.tile_pool(name="attn_sb", bufs=3))
        ps_rot = ctx.enter_context(tc.tile_pool(name="ps_rot", bufs=5, space="PSUM"))
        ps_acc = ctx.enter_context(tc.tile_pool(name="ps_acc", bufs=3, space="PSUM"))

        ones_sb = consts.tile([128, 128], BF16)
        nc.vector.memset(ones_sb, 1.0)
        eps_sb = consts.tile([128, 1], FP32)
        nc.vector.memset(eps_sb, EPS)
        ident = consts.tile([128, 128], BF16)
        make_identity(nc, ident)

        warm_ps = ps_acc.tile([128, 512], FP32, tag="pa")
        for _ in range(48):
            nc.tensor.matmul(warm_ps[:, 0:128], ones_sb, ones_sb[:, 0:128], start=True, stop=True)

        wq_sb = consts.tile([128, CT, 128], BF16)
        nc.sync.dma_start(out=wq_sb, in_=wq_d.rearrange("(a p) m -> p a m", p=128))
        wk_sb = consts.tile([128, CT, 128], BF16)
        nc.sync.dma_start(out=wk_sb, in_=wk_d.rearrange("(a p) m -> p a m", p=128))
        wv_sb = consts.tile([128, CT, 128], BF16)
        nc.sync.dma_start(out=wv_sb, in_=wv_d.rearrange("(a p) m -> p a m", p=128))
        nws_sb = consts.tile([128, 3], FP32)
        nc.sync.dma_start(out=nws_sb, in_=nws_d)
        wb_sb = consts.tile([128, 3], FP32)
        nc.sync.dma_start(out=wb_sb, in_=wb_d)
        tri_sb = consts.tile([128, 128], BF16)
        nc.sync.dma_start(out=tri_sb, in_=tri_d)

        xts = []
        for ci in range(CT):
            t = hpool.tile([128, TOK], BF16, tag=f"hT{ci}")
            xts.append(t)
        for j in range(NCH):
            sl = slice(j * 512, (j + 1) * 512)
            for ci in range(CT):
                nc.sync.dma_start(out=xts[ci][:, sl], in_=xt_d[ci * 128:(ci + 1) * 128, sl])

        rstd_b = stats.tile([128, TOK], BF16, tag="rstd_b")
        murstd_b = stats.tile([128, TOK], BF16, tag="murstd_b")
        mu_b = stats.tile([128, TOK], FP32, tag="mu_b")
        var_b = stats.tile([128, TOK], FP32, tag="var_b")
        for j in range(NCH):
            sl = slice(j * 512, (j + 1) * 512)
            ps_sum = ps_rot.tile([128, 512], FP32, tag="mm")
            ps_sq = ps_rot.tile([128, 512], FP32, tag="mm")
            for ci in range(CT):
                sq = tmp.tile([128, 512], BF16, tag="sq")
                nc.vector.tensor_mul(sq, xts[ci][:, sl], xts[ci][:, sl])
                nc.tensor.matmul(ps_sum, ones_sb, xts[ci][:, sl],
                                 start=(ci == 0), stop=(ci == CT - 1))
                nc.tensor.matmul(ps_sq, ones_sb, sq,
                                 start=(ci == 0), stop=(ci == CT - 1))
            nc.scalar.mul(out=mu_b[:, sl], in_=ps_sum, mul=1.0 / C)
            nc.scalar.mul(out=var_b[:, sl], in_=ps_sq, mul=1.0 / C)
            mu2 = tmp.tile([128, 512], BF16, tag="mu2")
            nc.vector.tensor_mul(mu2, mu_b[:, sl], mu_b[:, sl])
            nc.vector.tensor_sub(var_b[:, sl], var_b[:, sl], mu2)
        nc.scalar.activation(out=var_b, in_=var_b, func=AF.Ln, bias=eps_sb, scale=1.0)
        nc.scalar.activation(out=rstd_b, in_=var_b, func=AF.Exp, scale=-0.5)
        nc.vector.tensor_mul(murstd_b, mu_b, rstd_b)

        qt_sb = qkv.tile([128, TOK], BF16, tag="qt")
        kt_sb = qkv.tile([128, TOK], BF16, tag="kt")
        vt_sb = qkv.tile([128, TOK], BF16, tag="vt")
        wtriples = ((wq_sb, 0, qt_sb), (wk_sb, 1, kt_sb), (wv_sb, 2, vt_sb))

        def qkv_fold(tsb, idx, sl, ps):
            nc.scalar.copy(out=tsb[:, sl], in_=ps)
            nc.vector.tensor_mul(tsb[:, sl], tsb[:, sl], rstd_b[:, sl])
            nc.vector.scalar_tensor_tensor(
                tsb[:, sl], murstd_b[:, sl], nws_sb[:, idx:idx + 1], tsb[:, sl],
                op0=mybir.AluOpType.mult, op1=mybir.AluOpType.add)
            if use_beta:
                nc.vector.tensor_scalar_add(tsb[:, sl], tsb[:, sl],
                                            wb_sb[:, idx:idx + 1])

        def vt_build(tt):
            vt = vts[tt]
            ptv = ps_rot.tile([128, 128], BF16, tag="mm")
            nc.tensor.transpose(ptv, vt_sb[:, tt * 128:(tt + 1) * 128], ident)
            nc.scalar.copy(out=vt.rearrange("p (g c) -> p g c", g=2)[:, :, 0:64],
                           in_=ptv.rearrange("p (g c) -> p g c", g=2))
            nc.vector.memset(vt[:, 64:65], 1.0)
            nc.vector.memset(vt[:, 129:130], 1.0)

        vts = []
        for tt in range(TOK // 128):
            vt_tile = qkv.tile([128, 130], BF16, tag=f"v{tt}", name=f"v{tt}")
            vts.append(vt_tile)

        for wsb, idx, tsb in wtriples:
            pss = []
            for jl in range(4):
                p = ps_rot.tile([128, 512], FP32, tag="mm")
                pss.append(p)
            for ci in range(CT):
                for jl in range(4):
                    sl = slice(jl * 512, (jl + 1) * 512)
                    nc.tensor.matmul(pss[jl], wsb[:, ci, :], xts[ci][:, sl],
                                     start=(ci == 0), stop=(ci == CT - 1))
            for jl in range(4):
                qkv_fold(tsb, idx, slice(jl * 512, (jl + 1) * 512), pss[jl])
        for tt in range(16):
            vt_build(tt)

        def late_work():
            for wsb, idx, tsb in wtriples:
                for jp in range(2):
                    ps_a = ps_rot.tile([128, 512], FP32, tag="mm")
                    ps_b = ps_rot.tile([128, 512], FP32, tag="mm")
                    j0 = 4 + 2 * jp
                    sl_a = slice(j0 * 512, (j0 + 1) * 512)
                    sl_b = slice((j0 + 1) * 512, (j0 + 2) * 512)
                    for ci in range(CT):
                        nc.tensor.matmul(ps_a, wsb[:, ci, :], xts[ci][:, sl_a],
                                         start=(ci == 0), stop=(ci == CT - 1),
                                         skip_group_check=True)
                        nc.tensor.matmul(ps_b, wsb[:, ci, :], xts[ci][:, sl_b],
                                         start=(ci == 0), stop=(ci == CT - 1),
                                         skip_group_check=True)
                        yield
                    qkv_fold(tsb, idx, sl_a, ps_a)
                    qkv_fold(tsb, idx, sl_b, ps_b)
                    yield
            for tt in range(16, 32):
                vt_build(tt)
                yield
            ready_b1[0] = True

        if debug:
            nc.sync.dma_start(out=dbg_rstd, in_=rstd_b)
            nc.sync.dma_start(out=dbg_mur, in_=murstd_b)
            nc.sync.dma_start(out=dbg_qt, in_=qt_sb)
            nc.sync.dma_start(out=dbg_kt, in_=kt_sb)
            nc.sync.dma_start(out=dbg_v, in_=vts[0])

        scale = C ** -0.5

        def attn_group(b, hl, gate=None):
            hsl = slice(hl * 64, (hl + 1) * 64)
            vcol = slice(hl * 65, hl * 65 + 65)
            pend = []

            def _flush(item):
                pa_, q0_, koff_, pr_, c0_, st_, sp_ = item
                nc.tensor.matmul(pa_[:, c0_:] if c0_ else pa_,
                                 vts[koff_ // 128][:, vcol],
                                 pr_[:, c0_:] if c0_ else pr_,
                                 start=st_, stop=sp_, skip_group_check=True)
                if sp_:
                    asb = attn_sb_p.tile([65, 512], FP32, tag="asb")
                    nc.scalar.copy(out=asb, in_=pa_)
                    den = attn_sb_p.tile([1, 512], FP32, tag="den")
                    nc.gpsimd.dma_start(out=den, in_=asb[64:65, :])
                    rec = attn_sb_p.tile([64, 512], FP32, tag="rec")
                    nc.gpsimd.partition_broadcast(rec, den, channels=64)
                    recf = attn_sb_p.tile([64, 512], FP32, tag="recf")
                    nc.vector.reciprocal_approx_fast(recf, rec)
                    ao = attn_sb_p.tile([64, 512], BF16, tag="ao")
                    nc.vector.tensor_mul(ao, asb[0:64, :], recf)
                    nc.gpsimd.dma_start(out=out_d[hl * 64:(hl + 1) * 64, q0_:q0_ + QB],
                                         in_=ao)

            if gate is not None:
                while not gate[0]:
                    yield
            for j in range(T // QB):
                q0 = b * T + j * QB
                pa = ps_acc.tile([65, 512], FP32, tag="pa")
                nkt = 4 * (j + 1)
                for kt in range(nkt):
                    koff = b * T + kt * 128
                    d = kt - 4 * j
                    c0 = 128 * d if d > 0 else 0
                    ps = ps_rot.tile([128, 512], FP32, tag="mm")
                    nc.tensor.matmul(ps[:, c0:], kt_sb[hsl, koff:koff + 128],
                                     qt_sb[hsl, q0 + c0:q0 + QB],
                                     start=True, stop=True)
                    pr = probs_p.tile([128, 512], BF16, tag="pr")
                    nc.scalar.activation(out=pr[:, c0:], in_=ps[:, c0:],
                                         func=AF.Exp, scale=scale)
                    if d >= 0:
                        nc.vector.tensor_mul(pr[:, 128 * d:128 * (d + 1)],
                                             pr[:, 128 * d:128 * (d + 1)], tri_sb)
                    pend.append((pa, q0, koff, pr, c0, kt == 0, kt == nkt - 1))
                    if len(pend) > 4:
                        _flush(pend.pop(0))
                    yield
            while pend:
                _flush(pend.pop(0))
                yield

        ready_b1 = [False]
        lw = late_work()
        for b in range(B):
            gens = [attn_group(b, 0), attn_group(b, 1)] + ([lw] if b == 0 else [])
            while gens:
                for g in list(gens):
                    try:
                        next(g)
                    except StopIteration:
                        gens.remove(g)
    nc.compile()
    return nc


FP8 = mybir.dt.float8e4
f8 = ml_dtypes.float8_e4m3


def build_l2():
    nc = bacc.Bacc("TRN2", target_bir_lowering=False, debug=False, num_devices=NCORES)
    atq_d = nc.dram_tensor("atq", [128, CT, ROWS], BF16, kind="ExternalInput").ap()
    wpq_d = nc.dram_tensor("wpq", [128, CT, CT, 128], BF16, kind="ExternalInput").ap()
    xr_d = nc.dram_tensor("xrt", [128, CT, ROWS], FP32, kind="ExternalInput").ap()
    w1q_d = nc.dram_tensor("w1q", [128, HT, CT * 128], BF16, kind="ExternalInput").ap()
    w2q_d = nc.dram_tensor("w2q", [128, HT, C], BF16, kind="ExternalInput").ap()
    b1_d = nc.dram_tensor("b1r", [128, HT], FP32, kind="ExternalInput").ap()
    b2_d = nc.dram_tensor("b2c", [128, CT], FP32, kind="ExternalInput").ap()
    out_d = nc.dram_tensor("outT", [C, ROWS], FP32, kind="ExternalOutput").ap()

    with tile.TileContext(nc) as tc, contextlib.ExitStack() as ctx:
        consts = ctx.enter_context(tc.tile_pool(name="consts", bufs=1))
        persist = ctx.enter_context(tc.tile_pool(name="persist", bufs=1))
        wstream = ctx.enter_context(tc.tile_pool(name="wstream", bufs=4))
        tmp = ctx.enter_context(tc.tile_pool(name="tmp", bufs=2))
        small = ctx.enter_context(tc.tile_pool(name="small", bufs=1))
        ps_main = ctx.enter_context(tc.tile_pool(name="ps_main", bufs=6, space="PSUM"))
        ps_stats = ctx.enter_context(tc.tile_pool(name="ps_stats", bufs=1, space="PSUM"))

        ones_w = consts.tile([128, 128], BF16)
        nc.vector.memset(ones_w, 1.0)
        dummy_m = consts.tile([128, 512], BF16)
        nc.vector.memset(dummy_m, 0.0)
        eps_sb = consts.tile([128, 1], FP32)
        nc.vector.memset(eps_sb, EPS)

        warm_ps = ps_main.tile([128, 512], FP32, tag="mm")
        for _ in range(24):
            nc.tensor.matmul(warm_ps[:, 0:128], ones_w, ones_w[:, 0:128],
                             start=True, stop=True)

        atq_sb = consts.tile([128, CT, ROWS], BF16)
        nc.sync.dma_start(out=atq_sb, in_=atq_d)
        wpq_sb = []
        for n8 in range(CT):
            wpt = consts.tile([128, CT, 128], BF16, tag=f"wpq{n8}", name=f"wpq{n8}")
            eng = nc.sync if n8 % 2 == 0 else nc.gpsimd
            eng.dma_start(out=wpt, in_=wpq_d[:, n8])
            wpq_sb.append(wpt)
        xr_sb = consts.tile([128, CT, ROWS], FP32)
        nc.scalar.dma_start(out=xr_sb, in_=xr_d)
        b1_sb = consts.tile([128, HT], FP32)
        nc.scalar.dma_start(out=b1_sb, in_=b1_d)
        b2_sb = consts.tile([128, CT], FP32)
        nc.scalar.dma_start(out=b2_sb, in_=b2_d)
        w2_sb = consts.tile([128, HT, C], BF16)
        for g in range(8):
            nc.scalar.dma_start(out=w2_sb[:, g * 4:(g + 1) * 4],
                                in_=w2q_d[:, g * 4:(g + 1) * 4])

        x2_sb = persist.tile([128, CT, ROWS], FP32, tag="x2")
        ps_sum = ps_stats.tile([128, 512], FP32, tag="sum")
        ps_sq = ps_stats.tile([128, 512], FP32, tag="sq")
        for n8 in range(CT):
            pp = ps_main.tile([128, 512], FP32, tag="mm")
            for ci in range(CT):
                nc.tensor.matmul(pp, wpq_sb[n8][:, ci],
                                 atq_sb[:, ci], start=(ci == 0), stop=(ci == CT - 1),
                                 skip_group_check=True)
            nc.vector.tensor_add(x2_sb[:, n8], pp, xr_sb[:, n8])
            x2b = tmp.tile([128, 512], BF16, tag="x2b")
            nc.scalar.copy(out=x2b, in_=x2_sb[:, n8])
            sqb = tmp.tile([128, 512], BF16, tag="sqb")
            nc.vector.tensor_mul(sqb, x2b, x2b)
            nc.tensor.matmul(ps_sum, ones_w, x2b,
                             start=(n8 == 0), stop=(n8 == CT - 1),
                             skip_group_check=True)
            nc.tensor.matmul(ps_sq, ones_w, sqb,
                             start=(n8 == 0), stop=(n8 == CT - 1),
                             skip_group_check=True)

        fill_ps = ps_main.tile([128, 512], FP32, tag="mm")
        for _ in range(32):
            nc.tensor.matmul(fill_ps, ones_w, dummy_m, start=True, stop=True,
                             skip_group_check=True)

        mu = small.tile([128, 512], FP32, tag="mu")
        nc.scalar.mul(out=mu, in_=ps_sum, mul=1.0 / C)
        var = small.tile([128, 512], FP32, tag="var")
        nc.scalar.mul(out=var, in_=ps_sq, mul=1.0 / C)
        mu2 = small.tile([128, 512], FP32, tag="mu2")
        nc.vector.tensor_mul(mu2, mu, mu)
        nc.vector.tensor_sub(var, var, mu2)
        nc.scalar.activation(out=var, in_=var, func=AF.Ln, bias=eps_sb, scale=1.0)
        rstd = small.tile([128, 512], FP32, tag="rstd")
        nc.scalar.activation(out=rstd, in_=var, func=AF.Exp, scale=-0.5)
        murstd = small.tile([128, 512], FP32, tag="murstd")
        nc.vector.tensor_mul(murstd, mu, rstd)

        h2_sb = persist.tile([128, CT, ROWS], BF16, tag="h2")
        for ci in range(CT):
            th = tmp.tile([128, 512], FP32, tag="th")
            nc.vector.tensor_mul(th, x2_sb[:, ci], rstd)
            nc.vector.tensor_sub(h2_sb[:, ci], th, murstd)

        h1_sb = persist.tile([128, HT, ROWS], BF16, tag="h1")
        for ht in range(HT):
            w1t = wstream.tile([128, CT, 128], BF16, tag="w1t")
            weng = nc.sync if ht % 2 == 0 else nc.gpsimd
            weng.dma_start(out=w1t,
                           in_=w1q_d[:, ht, :].rearrange("p (a m) -> p a m", a=CT))
            ph = ps_main.tile([128, 512], FP32, tag="mm")
            for ci in range(CT):
                nc.tensor.matmul(ph, w1t[:, ci], h2_sb[:, ci],
                                 start=(ci == 0), stop=(ci == CT - 1),
                                 skip_group_check=True)
            nc.scalar.activation(out=h1_sb[:, ht], in_=ph,
                                 func=AF.Relu, bias=b1_sb[:, ht:ht + 1], scale=1.0)

        for pn in range(4):
            pos = []
            for k in range(2):
                po = ps_main.tile([128, 512], FP32, tag="mm")
                pos.append(po)
            for ht in range(HT):
                for k in range(2):
                    n8 = pn * 2 + k
                    nc.tensor.matmul(pos[k], w2_sb[:, ht, n8 * 128:(n8 + 1) * 128],
                                     h1_sb[:, ht], start=(ht == 0), stop=(ht == HT - 1),
                                     skip_group_check=True)
            for k in range(2):
                n8 = pn * 2 + k
                td = tmp.tile([128, 512], FP32, tag="td")
                nc.scalar.activation(out=td, in_=pos[k], func=AF.Identity,
                                     bias=b2_sb[:, n8:n8 + 1], scale=1.0)
                ot = tmp.tile([128, 512], FP32, tag="ot")
                nc.vector.tensor_add(ot, td, x2_sb[:, n8])
                nc.sync.dma_start(out=out_d[n8 * 128:(n8 + 1) * 128, :], in_=ot)
    nc.compile()
    return nc



def prep_l1_inputs(inputs):
    x = np.asarray(inputs["x"], np.float32).reshape(TOK, C)
    g1 = np.asarray(inputs["g1"], np.float32)
    beta1 = np.asarray(inputs["beta1"], np.float32)
    xt = np.ascontiguousarray(x.T).astype(bf16)
    wq = (g1[:, None] * np.asarray(inputs["Wq"], np.float32)).astype(bf16)
    wk = (g1[:, None] * np.asarray(inputs["Wk"], np.float32)).astype(bf16)
    wv = (g1[:, None] * np.asarray(inputs["Wv"], np.float32)).astype(bf16)
    tri = np.triu(np.ones((128, 128), np.float32)).astype(bf16)
    in_maps = []
    for c in range(NCORES):
        csl = slice(c * 128, (c + 1) * 128)
        nws = np.stack([-wq[:, csl].astype(np.float32).sum(0),
                        -wk[:, csl].astype(np.float32).sum(0),
                        -wv[:, csl].astype(np.float32).sum(0)], axis=1)
        wb = np.stack([wq[:, csl].astype(np.float32).T @ beta1,
                       wk[:, csl].astype(np.float32).T @ beta1,
                       wv[:, csl].astype(np.float32).T @ beta1], axis=1)
        in_maps.append({
            "xt": xt,
            "wq": np.ascontiguousarray(wq[:, csl]),
            "wk": np.ascontiguousarray(wk[:, csl]),
            "wv": np.ascontiguousarray(wv[:, csl]),
            "nws": np.ascontiguousarray(nws.astype(np.float32)),
            "wb": np.ascontiguousarray(wb.astype(np.float32)),
            "tri": tri,
        })
    return in_maps


def prep_l2_inputs(inputs, attn_t):
    attn_t = np.ascontiguousarray(np.asarray(attn_t, bf16))
    x = np.asarray(inputs["x"], np.float32).reshape(TOK, C)
    x = x + np.asarray(inputs["bp"], np.float32)[None, :]
    g2 = np.asarray(inputs["g2"], np.float32)
    beta2 = np.asarray(inputs["beta2"], np.float32)
    wp = np.asarray(inputs["Wp"], np.float32)
    w1 = np.asarray(inputs["W1"], np.float32)
    w2 = np.asarray(inputs["W2"], np.float32)
    wpq = np.ascontiguousarray(
        wp.astype(bf16).reshape(CT, 128, CT, 128).transpose(1, 2, 0, 3))
    w1g = (g2[:, None] * w1).astype(bf16)
    w1q = np.ascontiguousarray(
        w1g.reshape(CT, 128, HT, 128).transpose(1, 2, 0, 3).reshape(128, HT, CT * 128))
    w2q = np.ascontiguousarray(
        w2.astype(bf16).reshape(HT, 128, C).transpose(1, 0, 2))
    b1e = np.asarray(inputs["b1"], np.float32) + w1.T @ beta2
    b1r = np.ascontiguousarray(b1e.reshape(HT, 128).T)
    b2c = np.ascontiguousarray(
        np.asarray(inputs["b2"], np.float32).reshape(CT, 128).T)
    in_maps = []
    for c in range(NCORES):
        rsl = slice(c * ROWS, (c + 1) * ROWS)
        atq = np.ascontiguousarray(
            attn_t[:, rsl].reshape(CT, 128, ROWS).transpose(1, 0, 2))
        xrt = np.ascontiguousarray(
            x[rsl, :].T.reshape(CT, 128, ROWS).transpose(1, 0, 2))
        in_maps.append({
            "atq": atq,
            "wpq": wpq,
            "xrt": xrt,
            "w1q": w1q,
            "w2q": w2q,
            "b1r": b1r,
            "b2c": b2c,
        })
    return in_maps


_CACHE = {}


def _get_programs(use_beta):
    key = ("progs", bool(use_beta))
    if key not in _CACHE:
        nc1 = build_l1(use_beta=use_beta)
        nc2 = build_l2()
        _CACHE[key] = (nc1, nc2)
    return _CACHE[key]


def kernel(**inputs):
    from concourse.bass_utils import run_bass_kernel_spmd

    inputs = {k: np.asarray(v) for k, v in inputs.items()}
    use_beta = bool(np.any(np.asarray(inputs["beta1"], np.float32) != 0.0))
    nc1, nc2 = _get_programs(use_beta)
    core_ids = list(range(NCORES))

    r1 = run_bass_kernel_spmd(nc1, prep_l1_inputs(inputs), core_ids)
    attn_t = np.concatenate(
        [np.asarray(r1.results[c]["attn_out"]) for c in range(NCORES)], axis=0)

    r2 = run_bass_kernel_spmd(nc2, prep_l2_inputs(inputs, attn_t), core_ids)
    out = np.concatenate(
        [np.asarray(r2.results[c]["outT"]).T for c in range(NCORES)], axis=0)
    return np.ascontiguousarray(out.reshape(B, T, C).astype(np.float32))

